# revision 33
# baseline (speedup 1.0000x reference)
"""Causal self-attention TRN2 kernel.

Problem: B=4, T=2048, C=1024, H=16 heads, Dh=64, fp32 I/O.

Sharding: 8 cores = 4 batches x 2 head-groups (8 heads each). Each core
computes QKV projection for its head-group, causal attention, and a partial
output projection; the host sums the two partials per batch and adds b_out.

v3 design (per-core):
  - Q/K projection in fp16 (fp8 is too lossy ahead of exp: measured 3.3e-2
    rel err vs the 2e-2 budget).
  - V stored fp8 as key-block PAIRS [128, 2, 8*66] so the AV matmul for
    unmasked kb-pairs runs as ONE DoubleRow matmul (contraction 256 keys,
    512 cols instead of 1024). exp emits P directly in fp8 for those pairs.
    Masked (diagonal) pairs keep fp16 P (mask multiply) with plain per-block
    AV matmuls against the fp8 V planes.
  - AV pairs are deferred TWO kb-pairs behind their exp so the in-order PE
    never waits on the ACT engine.
  - softmax row-sum reciprocals use reciprocal_approx_fast (the bit-exact
    nc.vector.reciprocal is ~6 cycles/elem and was ~100us of DVE time);
    normalize multiplies run off the critical path (per-head GPSIMD bcast,
    fused descale in scalar_tensor_tensor).
  - QK PSUM evictions alternate DVE/GPSIMD to halve the DVE load.
"""

import numpy as np

import concourse.bacc as bacc
import concourse.mybir as mybir
import concourse.tile as tile
from concourse import bass_utils

F32 = mybir.dt.float32
F16 = mybir.dt.float16
F8 = mybir.dt.float8e4
AF = mybir.ActivationFunctionType
ALU = mybir.AluOpType
DR = mybir.MatmulPerfMode.DoubleRow

B, T, C = 4, 2048, 1024
H, DH = 16, 64
HPC = 512          # head dims per core (8 heads x 64)
NHP = 4            # head pairs per core
NC_CHUNKS = C // 128   # 8 fp16 contraction chunks (Q/K proj)
NC2 = 4            # fp8 DoubleRow pair-chunks (V proj)
NTS = T // 512     # 4 t-chunks of 512
NTB = T // 128     # 16 t-blocks of 128
SCALE = 1.0 / np.sqrt(DH)
S_V = 32.0         # scale of V values as stored in fp8 SBUF tiles
RECIP_FAST = False  # reciprocal_approx_fast vs bit-exact reciprocal

_cache = {}


def _build(loop_iters=1, loop_phases=(1, 2, 3), ablate=None):
    key = ("nc", loop_iters, tuple(loop_phases), ablate)
    if key in _cache:
        return _cache[key]
    nc = bacc.Bacc(trn_type="TRN2", target_bir_lowering=False, debug=False)

    xt = nc.dram_tensor("xt", [C, T], F16, kind="ExternalInput").ap()
    wq = nc.dram_tensor("wq", [C, HPC], F16, kind="ExternalInput").ap()
    wk = nc.dram_tensor("wk", [C, HPC], F16, kind="ExternalInput").ap()
    wv = nc.dram_tensor("wv", [C, HPC], F16, kind="ExternalInput").ap()
    wo = nc.dram_tensor("wo", [HPC, C], F16, kind="ExternalInput").ap()
    bqk = nc.dram_tensor("bqk", [128, 2 * NHP], F32, kind="ExternalInput").ap()
    bv = nc.dram_tensor("bv", [128, HPC], F16, kind="ExternalInput").ap()
    masks = nc.dram_tensor("masks", [128, 1280], F16, kind="ExternalInput").ap()
    yt = nc.dram_tensor("yt", [C, T], F32, kind="ExternalOutput").ap()

    def pairv(t):
        # [128, 2*w] tile -> [128, 2, w] DoubleRow plane view
        return t[:].rearrange("p (i m) -> p i m", i=2)

    with tile.TileContext(nc) as tc:
        with (
            tc.tile_pool(name="wp", bufs=1) as wp,          # persistent weights/consts
            tc.tile_pool(name="big", bufs=1) as big,        # QT/KT/V/OT persistent
            tc.tile_pool(name="xs", bufs=1) as xs,          # streamed xT chunks
            tc.tile_pool(name="ev", bufs=3) as ev,          # small sbuf staging
            tc.tile_pool(name="ps", bufs=1, space="PSUM") as ps,
        ):
            # ---- persistent loads (outside the timing loop) ----
            wq_t, wk_t, wv_t = [], [], []
            for c in range(NC_CHUNKS):
                wqc = wp.tile([128, HPC], F16, name=f"wq{c}", tag=f"wq{c}")
                nc.sync.dma_start(wqc[:], wq[c * 128:(c + 1) * 128, :])
                wq_t.append(wqc)
                wkc = wp.tile([128, HPC], F16, name=f"wk{c}", tag=f"wk{c}")
                nc.sync.dma_start(wkc[:], wk[c * 128:(c + 1) * 128, :])
                wk_t.append(wkc)
            for c in range(NC_CHUNKS):
                wvc = wp.tile([128, HPC], F16, name=f"wv{c}", tag=f"wv{c}")
                nc.sync.dma_start(wvc[:], wv[c * 128:(c + 1) * 128, :])
                wv_t.append(wvc)
            bqk_t = wp.tile([128, 2 * NHP], F32, name="bqk_t", tag="bqk")
            nc.sync.dma_start(bqk_t[:], bqk)
            bv_t = wp.tile([128, HPC], F16, name="bv_t", tag="bv")
            nc.sync.dma_start(bv_t[:], bv)
            mask_t = wp.tile([128, 1280], F16, name="mask_t", tag="mask")
            nc.sync.dma_start(mask_t[:], masks)
            wo_t = []
            for hp in range(NHP):
                woc = wp.tile([128, C], F16, name=f"wo{hp}", tag=f"wo{hp}")
                nc.sync.dma_start(woc[:], wo[hp * 128:(hp + 1) * 128, :])
                wo_t.append(woc)
            if ablate is not None:
                pconst = wp.tile([128, 1024], F16, name="pconst", tag="pconst")
                nc.vector.memset(pconst[:], 2.0 ** -11)
                pconst8 = wp.tile([128, 1024], F8, name="pconst8", tag="pconst8")
                nc.vector.memset(pconst8[:], 2.0 ** -7)
                bcdummy2 = wp.tile([64, 2048], F32, name="bcdummy2", tag="bcdummy")
                nc.vector.memset(bcdummy2[:], 1.0)

            qt_t = [big.tile([128, T], F16, name=f"qt{i}", tag=f"qt{i}") for i in range(NHP)]
            kt_t = [big.tile([128, T], F16, name=f"kt{i}", tag=f"kt{i}") for i in range(NHP)]
            # fp16 V (S_V-scaled, ones col = S_V) for masked/diagonal AV.
            # 66-wide head slots so the fp8 copy below is fully contiguous.
            v16_t = [big.tile([128, 8, 66], F16, name=f"v16_{i}", tag=f"v16_{i}")
                     for i in range(NTB)]
            # fp8 V in kb-PAIR tiles [128 keys, 2 planes, 8 heads x 66] for
            # the DoubleRow AV on unmasked pairs
            v_t = [big.tile([128, 2, 8 * 66], F8, name=f"v{i}", tag=f"v{i}")
                   for i in range(NTB // 2)]
            ot_t = [big.tile([128, T], F16, name=f"ot{i}", tag=f"ot{i}") for i in range(NHP)]

            def vplane16(kb, h):
                # [128, 65] fp16 V for key-block kb, head h (64 dims + ones)
                return v16_t[kb][:, h, 0:65]

            def vpair(m, h):
                # [128, 2, 65] DoubleRow view for kb pair (2m, 2m+1), head h
                return v_t[m][:, :, h * 66:h * 66 + 65]

            def body(phases=(1, 2, 3), real=False):
                if tuple(phases) == (1, 2, 3):
                    # interleaved: QK for head-pairs 1-3 emitted as quanta
                    # inside phase 2's kb-pair stream to fill PE stall bubbles
                    xrow = phase1_load()
                    phase1_qk(xrow, 0)
                    phase1_v(xrow)
                    queues = {hp: qk_quanta(xrow, hp) for hp in (1, 2, 3)}
                    phase2(real=real, qk_queues=queues)
                    phase3()
                    return
                if 1 in phases:
                    phase1()
                if 2 in phases:
                    phase2(real=real)
                if 3 in phases:
                    phase3()

            def phase1_load():
                xrow = []
                for c in range(NC_CHUNKS):
                    xr = xs.tile([128, T], F16, name=f"x_{c}", tag=f"x{c}")
                    nc.sync.dma_start(xr[:], xt[c * 128:(c + 1) * 128, :])
                    xrow.append(xr)
                return xrow

            def qk_quanta(xrow, hp):
                # list of zero-arg closures: 8 matmuls + 1 eviction per (ts, q/k)
                quanta = []
                for ts in range(NTS):
                    xc = [xr[:, ts * 512:(ts + 1) * 512] for xr in xrow]
                    for which in (0, 1):
                        w_t = wq_t if which == 0 else wk_t
                        dst = (qt_t if which == 0 else kt_t)[hp]
                        bias_col = hp if which == 0 else NHP + hp
                        p = ps.tile([128, 512], F32,
                                    name=f"pqk_{hp}_{ts}_{which}", tag="st", bufs=3)

                        def mk_mm(p=p, w_t=w_t, xc=xc, hp=hp, c=0):
                            return lambda: nc.tensor.matmul(
                                p[:], w_t[c][:, hp * 128:(hp + 1) * 128], xc[c],
                                start=(c == 0), stop=(c == NC_CHUNKS - 1))
                        for c in range(NC_CHUNKS):
                            quanta.append(mk_mm(c=c))

                        def mk_ev(p=p, dst=dst, ts=ts, bias_col=bias_col):
                            return lambda: nc.vector.tensor_scalar_add(
                                dst[:, ts * 512:(ts + 1) * 512], p[:],
                                bqk_t[:, bias_col:bias_col + 1])
                        quanta.append(mk_ev())
                return quanta

            def phase1_qk(xrow, hp):
                for q in qk_quanta(xrow, hp):
                    q()

            def phase1_v(xrow):
                for ts in range(NTS):
                    xc = [xr[:, ts * 512:(ts + 1) * 512] for xr in xrow]
                    for tb in range(4):
                        tbg = ts * 4 + tb
                        pv = ps.tile([128, 512], F32, name=f"pv_{tbg}", tag="st", bufs=3)
                        for c in range(NC_CHUNKS):
                            nc.tensor.matmul(
                                pv[:], xc[c][:, tb * 128:(tb + 1) * 128], wv_t[c][:],
                                start=(c == 0), stop=(c == NC_CHUNKS - 1),
                            )
                        d16 = v16_t[tbg]
                        # V16 = S_V * (pv + bv) ; bv host-scaled by S_V
                        nc.vector.scalar_tensor_tensor(
                            d16[:, :, 0:64],
                            pv[:].rearrange("p (h d) -> p h d", h=8),
                            S_V,
                            bv_t[:].rearrange("p (h d) -> p h d", h=8),
                            ALU.mult, ALU.add)
                        # ones column is S_V so the AV row-sum comes out as
                        # S_V*sum(P): its reciprocal then absorbs the S_V
                        # scaling, making the normalize a plain tensor_mul.
                        # (covers the 66th pad col too, keeping it finite)
                        nc.vector.memset(d16[:, :, 64:66], S_V)
                        # contiguous fp8 copy (incl. ones col) for DoubleRow AV
                        d8 = v_t[tbg // 2][:, tbg % 2].rearrange(
                            "p (h x) -> p h x", x=66)
                        nc.vector.tensor_copy(d8[:], d16[:])

            def phase1():
                xrow = phase1_load()
                for hp in range(NHP):
                    phase1_qk(xrow, hp)
                phase1_v(xrow)

            def phase2(real=False, qk_queues=None):
                if not real and ablate in ("mm512", "mmst", "mmav", "mmavdr"):
                    # PE micro-benchmarks: 640 matmuls in phase-2's slot
                    for i in range(640):
                        st = ps.tile([128, 512], F32, name=f"mb_{i}", tag="st", bufs=3)
                        if ablate == "mm512":
                            nc.tensor.matmul(st[:], kt_t[0][:, 0:128], pconst[:, 0:512],
                                             start=True, stop=True)
                        elif ablate == "mmst":
                            nc.tensor.matmul(st[:], kt_t[0][0:64, 0:128],
                                             qt_t[0][0:64, 0:512], start=True, stop=True)
                        elif ablate == "mmavdr":
                            nc.tensor.matmul(st[0:65, :], vpair(i % 8, 0),
                                             pairv(pconst8),
                                             start=True, stop=True, perf_mode=DR)
                        else:  # mmav
                            nc.tensor.matmul(st[0:65, :], vplane16(i % 16, 0),
                                             pconst[:, 0:512], start=True, stop=True)
                    return
                # ---- phase 2: causal attention ----
                use_abl = (not real) and ablate in ("nonorm", "dumbc")

                def do_evict(ot, hp, off, j, h, state):
                    # prompt PSUM-freeing evict + recip slice; bcast/mul batched per head
                    oraw = ev.tile([65, 512], F32, name=f"or_{h}_{j}", tag="oraw", bufs=5)
                    nc.vector.tensor_copy(oraw[:], ot[:])
                    if use_abl and ablate == "nonorm":
                        nc.vector.tensor_copy(
                            ot_t[hp][off:off + 64, j * 512:(j + 1) * 512], oraw[0:64, :])
                        return
                    if RECIP_FAST:
                        nc.vector.reciprocal_approx_fast(
                            state["recip"][:, j * 512:(j + 1) * 512], oraw[64:65, :])
                    else:
                        nc.vector.reciprocal(
                            state["recip"][:, j * 512:(j + 1) * 512], oraw[64:65, :])
                    state["oraw"][j] = oraw

                def flush_head(hp, off, h, state):
                    if use_abl and ablate == "nonorm":
                        return
                    if use_abl and ablate == "dumbc":
                        bc = bcdummy2
                    else:
                        bc = ev.tile([64, 2048], F32, name=f"bch_{h}", tag="bcs", bufs=2)
                        nc.gpsimd.partition_broadcast(bc[:], state["recip"][:])
                    for j, oraw in state["oraw"].items():
                        # all-SBUF operands: runs on GPSIMD (Pool) to keep
                        # DVE free for the PSUM evictions. The 1/S_V descale
                        # is already folded into recip via the ones column.
                        nc.gpsimd.tensor_mul(
                            ot_t[hp][off:off + 64, j * 512:(j + 1) * 512],
                            oraw[0:64, :], bc[:, j * 512:(j + 1) * 512])

                norm_q = []
                pending = []  # deferred AV pairs (depth 2)

                def emit_av(keep):
                    while len(pending) > keep:
                        pot, ph, pm, pkb0, pkb1, pp, pw0, pw1, pnkb, pdr = pending.pop(0)
                        if pdr:
                            nc.tensor.matmul(
                                pot[0:65, :], vpair(pm, ph),
                                pairv(pp)[:, :, 0:512],
                                start=(pkb0 == 0), stop=(pkb1 == pnkb - 1),
                                perf_mode=DR)
                        else:
                            for i, (kb, w) in enumerate(((pkb0, pw0), (pkb1, pw1))):
                                nc.tensor.matmul(
                                    pot[0:65, 512 - w:512], vplane16(kb, ph),
                                    pp[:, i * pw0:i * pw0 + w],
                                    start=(kb == 0), stop=(kb == pnkb - 1),
                                )

                head_state = {}
                for h in range(8):
                    hp, off = h // 2, 64 * (h % 2)
                    if qk_queues:
                        # this head's QT/KT must be complete: drain stragglers
                        for hp2 in range(1, hp + 1):
                            while qk_queues.get(hp2):
                                qk_queues[hp2].pop(0)()
                    head_state[h] = {
                        "recip": ev.tile([1, 2048], F32, name=f"rch_{h}", tag="recip", bufs=2),
                        "oraw": {},
                    }
                    for j in range(NTS):
                        nkb = 4 * j + 4
                        ot = ps.tile([65, 512], F32, name=f"ot_{h}_{j}", tag="ot", bufs=2)
                        for m in range(nkb // 2):   # kb pairs
                            kb0, kb1 = 2 * m, 2 * m + 1
                            masked = kb1 >= 4 * j
                            if masked:
                                # diagonal blocks: restrict to the valid query
                                # tail q in [128r, 512) of this j-chunk
                                r0 = kb0 - 4 * j  # 0 or 2
                                w0, w1 = 512 - 128 * r0, 512 - 128 * (r0 + 1)
                                packoff = 0 if r0 == 0 else 896
                            else:
                                w0 = w1 = 512
                            q0 = j * 512
                            st = ps.tile([128, 1024], F32, name=f"st_{h}_{j}_{m}", tag="st", bufs=3)
                            for i, (kb, w) in enumerate(((kb0, w0), (kb1, w1))):
                                nc.tensor.matmul(
                                    st[:, i * w0:i * w0 + w],
                                    kt_t[hp][off:off + 64, kb * 128:(kb + 1) * 128],
                                    qt_t[hp][off:off + 64, q0 + 512 - w:q0 + 512],
                                    start=True, stop=True,
                                )
                            wt = w0 + w1
                            if (not real) and ablate == "noexp":
                                pending.append((ot, h, m, kb0, kb1, pconst, w0, w1, nkb, False))
                            elif masked:
                                praw = ev.tile(
                                    [128, 1024], F16, name=f"pr_{h}_{j}_{m}", tag="praw", bufs=2)
                                p16 = ev.tile(
                                    [128, 1024], F16, name=f"p_{h}_{j}_{m}", tag="p", bufs=3)
                                nc.scalar.activation(
                                    praw[:, 0:wt], st[:, 0:wt], AF.Exp, scale=SCALE)
                                nc.vector.tensor_mul(
                                    p16[:, 0:wt], praw[:, 0:wt],
                                    mask_t[:, packoff:packoff + wt]
                                )
                                pending.append((ot, h, m, kb0, kb1, p16, w0, w1, nkb, False))
                            else:
                                p8 = ev.tile(
                                    [128, 1024], F8, name=f"p8_{h}_{j}_{m}", tag="p8", bufs=3)
                                nc.scalar.activation(
                                    p8[:, 0:wt], st[:, 0:wt], AF.Exp, scale=SCALE)
                                pending.append((ot, h, m, kb0, kb1, p8, w0, w1, nkb, True))
                            emit_av(2)
                            if qk_queues:
                                for hp2 in (1, 2, 3):
                                    if qk_queues.get(hp2):
                                        qk_queues[hp2].pop(0)()
                                        break
                        norm_q.append((ot, hp, off, j, h))
                        if len(norm_q) >= 3:
                            e = norm_q.pop(0)
                            do_evict(*e, head_state[e[4]])
                    # head boundary: flush deferred AV, drain evicts,
                    # then batched bcast+muls
                    emit_av(0)
                    for e in norm_q:
                        do_evict(*e, head_state[e[4]])
                    norm_q = []
                    flush_head(hp, off, h, head_state[h])
                    del head_state[h]
                emit_av(0)

            def phase3():
                # ---- phase 3: output projection (partial) ----
                for cc in range(C // 128):
                    for qs in range(NTS):
                        py = ps.tile([128, 512], F32, name=f"py_{cc}_{qs}", tag="st", bufs=3)
                        for hp in range(NHP):
                            nc.tensor.matmul(
                                py[:],
                                wo_t[hp][:, cc * 128:(cc + 1) * 128],
                                ot_t[hp][:, qs * 512:(qs + 1) * 512],
                                start=(hp == 0), stop=(hp == NHP - 1),
                            )
                        ys = ev.tile([128, 512], F32, name=f"ys_{cc}_{qs}", tag="ys", bufs=3)
                        nc.vector.tensor_copy(ys[:], py[:])
                        nc.sync.dma_start(
                            yt[cc * 128:(cc + 1) * 128, qs * 512:(qs + 1) * 512], ys[:])

            if loop_iters > 1:
                if tuple(loop_phases) != (1, 2, 3) or ablate is not None:
                    body(real=True)  # populate intermediates once
                with tc.For_i(0, loop_iters, 1):
                    body(tuple(loop_phases))
            else:
                body()

    nc.compile()
    _cache[key] = nc
    return nc


def _make_masks():
    # packed diagonal masks: pack0 = [tril 512 | tril 384], pack1 = [tril 256 | tril 128]
    kk = np.arange(128)[:, None]
    m = np.zeros((128, 1280), dtype=np.float16)
    off = 0
    for w in (512, 384, 256, 128):
        qq = np.arange(w)[None, :]
        m[:, off:off + w] = (kk <= qq).astype(np.float16)
        off += w
    return m


def build_in_maps(x, W_qkv, b_qkv, W_out):
    masks = _make_masks()
    in_maps = []
    for core in range(8):
        b, g = core // 2, core % 2
        sl = slice(g * HPC, (g + 1) * HPC)
        bq_c = b_qkv[0 * C:1 * C][sl]
        bk_c = b_qkv[1 * C:2 * C][sl]
        bv_c = b_qkv[2 * C:3 * C][sl]
        xb = np.ascontiguousarray(x[b].T)
        in_maps.append(dict(
            xt=xb.astype(np.float16),
            wq=W_qkv[:, 0 * C:1 * C][:, sl].astype(np.float16),
            wk=W_qkv[:, 1 * C:2 * C][:, sl].astype(np.float16),
            wv=W_qkv[:, 2 * C:3 * C][:, sl].astype(np.float16),
            wo=W_out[sl, :].astype(np.float16),
            bqk=np.concatenate(
                [bq_c.reshape(NHP, 128).T, bk_c.reshape(NHP, 128).T], axis=1
            ).astype(np.float32),
            bv=(np.tile(bv_c[None, :], (128, 1)) * S_V).astype(np.float16),
            masks=masks,
        ))
    return in_maps


def kernel(x, W_qkv, b_qkv, W_out, b_out):
    x = np.asarray(x, dtype=np.float32)
    W_qkv = np.asarray(W_qkv, dtype=np.float32)
    b_qkv = np.asarray(b_qkv, dtype=np.float32)
    W_out = np.asarray(W_out, dtype=np.float32)
    b_out = np.asarray(b_out, dtype=np.float32)

    nc = _build()
    in_maps = build_in_maps(x, W_qkv, b_qkv, W_out)

    res = bass_utils.run_bass_kernel_spmd(nc, in_maps, core_ids=list(range(8)))
    out = np.zeros((B, T, C), dtype=np.float32)
    for core in range(8):
        b = core // 2
        out[b] += res.results[core]["yt"].T
    out += b_out[None, None, :]
    return out
